# revision 48
# baseline (speedup 1.0000x reference)
"""CRPS loss kernel for Trainium2 (8 NeuronCores, axon-tunneled).

reference semantics:
  preds: [B=8, N=16, C=4, H=128, W=256] f32, gt: [B, C, H, W] f32
  term1 = mean_i |preds_i - gt|            (per point)
  term2 = sum_{i,j} |p_i - p_j| / (2N(N-1))
  out   = mean(term1 - term2)  (scalar f32)

Sharding: batch-parallel, core b handles batch element b (131072 points).

The 17 values per point (gt + 16 members) need |.| of 136 pair
differences (16 vs-gt + 120 member pairs). Engine plan per core:

 * TensorE path (points with c < PEF): a constant [17, 128] +1/-1
   stationary matrix turns one matmul into 128 pair-differences per
   point (16 term1 + 112 term2 columns; the 8 pairs (k, k+8) are left
   out to fit M=128). Moving data is streamed N-major; 4 copies of the
   stationary sit at partition 32g so row-group matmuls run
   concurrently. Each round fills a [128, 1024] 2-bank PSUM slot (two
   512-col matmuls on paired groups); a 4-slot ring keeps TensorE ~2
   slots ahead. ScalarE (activation Abs + accum_out) and VectorE
   (custom |x| + accum op) split the rounds so both consume in
   parallel at ~93% occupancy.
   The 8 leftover pairs come from the spatial-major copy via one big
   fused ABS_DIFF_ACC instruction.
 * SBUF path (points with c >= PEF): all 136 pairs via the fused
   ABS_DIFF_ACC custom DVE op (native ABSOLUTE_DIFF alu op + ADD
   accumulate) on slot-offset slices of the spatial-major tile.

Per-instruction partial sums land in separate columns of a [128, ncols]
f32 accumulator; the host finishes the reduction in float64. For PSUM
columns the partition index is the PAIR index (rows 0-15 = term1), for
SBUF columns it is spatial.

Inputs are cast to fp16 on the host (DVE tensor ops hit 2x mode, DMA
halves); all accumulation is fp32. Measured end-to-end scalar error vs
the f32 reference is ~1e-6 (noise cancels over 142M terms).
"""

import re
import sys

if "/opt/trn_rl_repo" not in sys.path:
    sys.path.insert(0, "/opt/trn_rl_repo")

from contextlib import ExitStack

import numpy as np

import concourse.tile as tile
from concourse import bacc, mybir
from concourse import dve_ops
from concourse.bass_utils import run_bass_kernel_spmd
from concourse.dve_spec import AluOp, Bin, Spec, Src0, Src1, Zero, maxx
from concourse.dve_ops import OPS, CUSTOM_DVE_SPECS, _SUB_OPCODE_FOR_NAME, DveOp

B, N, C, H, W = 8, 16, 4, 128, 256
CHW = C * H * W          # 131072 spatial points per batch element
P = 128                  # SBUF partitions
FP = CHW // P            # 1024 points per partition
SLOTS = N + 1            # gt + 16 members
XW = SLOTS * FP
CHUNK = 16384            # max free-dim per SBUF-path instruction

DT = np.float16
DT_MY = mybir.dt.float16
F32 = mybir.dt.float32

PEF = 512                # cols per partition routed through the TensorE path
MMF = 512                # moving free-dim per matmul
GROUPS = 4               # concurrent row-group matmuls
RND = 2 * MMF            # points consumed per PSUM round (2 groups/round)
PSUM_BUFS = 4

# leftover pairs not in the matmul: (slot k, slot k+8) for k=1..8
LEFT_D = 8

# measured per-round consumer costs (ns) to split rounds ACT vs DVE
_COST_ACT_ROUND = 1557.0
_COST_DVE_ROUND = 1390.0

_cache = {}


def _register_op(name, spec):
    for op in OPS:
        if op.name == name:
            return op
    op = DveOp(name, spec, subdim=False, uops_sha={})
    OPS.append(op)
    CUSTOM_DVE_SPECS[name] = spec
    _SUB_OPCODE_FOR_NAME[name] = dve_ops._CUSTOM_DVE_ROW_BASE + len(OPS) - 1
    for ver in ("v3", "v4"):
        try:
            op.compile(ver)
        except ValueError as e:
            m = re.search(r":\s*([0-9a-f]{8,32})\s*≠", str(e))
            if not m:
                raise
            op.uops_sha[ver] = m.group(1)
            op.compile(ver)
    return op


def _abs_diff_acc():
    """out = |in0 - in1|; accum_out = sum(out) along free dim."""
    return _register_op(
        "ABS_DIFF_ACC_CRPS",
        Spec(
            body=Bin(AluOp.ABSOLUTE_DIFF, Src0, Src1),
            accum=AluOp.ADD,
            reference=lambda in0, in1, s0, s1, imm2: np.abs(
                np.asarray(in0, np.float32) - np.asarray(in1, np.float32)
            ),
        ),
    )


def _abs_acc():
    """out = |in0|; accum_out = sum(out) along free dim (PSUM consumer)."""
    return _register_op(
        "ABS_ACC_CRPS",
        Spec(
            body=maxx(Src0, Zero - Src0),
            accum=AluOp.ADD,
            reference=lambda in0, in1, s0, s1, imm2: np.abs(
                np.asarray(in0, np.float32)
            ),
        ),
    )


def _pair_columns():
    """Stationary matrix columns: (pos_slot, neg_slot). First 16 = term1."""
    cols = [(0, d) for d in range(1, SLOTS)]
    for d in range(1, N):
        if d == LEFT_D:
            continue
        for k in range(1, SLOTS - d):
            cols.append((k, k + d))
    assert len(cols) == 128
    return cols


def _sbuf_chunks():
    """(kind, d, k0, kn) slot-window work units over cols [PEF, FP).

    kind 0: gt pairs (slot 0, slot d) — k0/kn unused (single window).
    kind 1: member pairs (slot k, slot k+d) for k in [k0, k0+kn).
    Chunks are whole slot windows; window width w = FP - PEF.
    """
    out = []
    w = FP - PEF
    if w <= 0:
        return out
    kmax = max(1, min(CHUNK // w, SLOTS - 2))
    for d in range(1, SLOTS):
        out.append((0, d, 0, 1))
        if d < SLOTS - 1:
            nk = SLOTS - 1 - d
            for k0 in range(1, 1 + nk, kmax):
                out.append((1, d, k0, min(kmax, 1 + nk - k0)))
    return out


def _build():
    fused = _abs_diff_acc()
    absacc = _abs_acc()
    nc = bacc.Bacc("TRN2", target_bir_lowering=False, debug=False, num_devices=8)

    n_rounds = (P * PEF) // RND
    x_d = nc.dram_tensor("x", [P, SLOTS, FP], DT_MY, kind="ExternalInput").ap()
    w_d = nc.dram_tensor("w", [P, P], DT_MY, kind="ExternalInput").ap()
    y_d = (
        nc.dram_tensor("y", [P, 32 * PEF], DT_MY, kind="ExternalInput").ap()
        if n_rounds
        else None
    )

    sb = _sbuf_chunks()
    # columns: [psum rounds][leftover d=8][sbuf chunks]; each engine gets its
    # own accumulator tensor so their accum writes share no dependency
    ncols = n_rounds + 1 + len(sb)
    acc_d = nc.dram_tensor("acc", [P, ncols], F32, kind="ExternalOutput").ap()
    acc2_d = nc.dram_tensor("acc2", [P, ncols], F32, kind="ExternalOutput").ap()

    # round -> consumer engine, balancing measured costs. DVE's SBUF-path
    # work needs X (which lands ~20us in), so DVE takes the FIRST rounds
    # and ACT everything after.
    left_cost = (LEFT_D * PEF) / 0.96 + 300.0
    w_sb = FP - PEF
    sbuf_cost = sum(kn * w_sb for _, _, _, kn in sb) / 0.96 + len(sb) * 160.0
    dve_fixed = left_cost + sbuf_cost
    n_act = n_rounds
    if n_rounds:
        n_act = max(0, min(n_rounds, round((_COST_DVE_ROUND * n_rounds + dve_fixed)
                                           / (_COST_ACT_ROUND + _COST_DVE_ROUND))))
    act_rounds = set(range(n_rounds - n_act, n_rounds))

    with tile.TileContext(nc) as tc, ExitStack() as ctx:
        xpool = ctx.enter_context(tc.tile_pool(name="x", bufs=1))
        wpool = ctx.enter_context(tc.tile_pool(name="w", bufs=1))
        ypool = ctx.enter_context(tc.tile_pool(name="y", bufs=4))
        pspool = ctx.enter_context(tc.tile_pool(name="ps", bufs=PSUM_BUFS, space="PSUM"))
        spool = ctx.enter_context(tc.tile_pool(name="scratch", bufs=2))
        apool = ctx.enter_context(tc.tile_pool(name="ascratch", bufs=2))
        dfpool = ctx.enter_context(tc.tile_pool(name="dfscratch", bufs=2))
        lpool = ctx.enter_context(tc.tile_pool(name="lscratch", bufs=2))
        accpool = ctx.enter_context(tc.tile_pool(name="acc", bufs=1))

        Wt = wpool.tile([P, P], DT_MY)
        nc.sync.dma_start(Wt[:], w_d[:])
        acc = accpool.tile([P, ncols], F32)
        acc2 = accpool.tile([P, ncols], F32)
        # explicit zero bias for Abs activations: the implicit const AP's
        # init otherwise lands behind the big X transfer in queue order,
        # stalling ACT's first instruction ~15us
        zb = accpool.tile([P, 1], F32, name="zb")
        nc.vector.memset(zb[:], 0.0)
        nc.vector.memset(acc[:], 0.0)
        nc.scalar.memzero(acc2[:])

        col = 0
        meta = []  # per-col: "psum" | 0 (term1) | 1 (term2)

        # --- TensorE path ---
        # y chunks stream ahead of the matmuls; the big X transfer is issued
        # interleaved late (leftover/SBUF work needs it last) so it doesn't
        # steal HBM bandwidth from the critical y stream. Chunk DMAs
        # alternate between the HWDGE (sync) and SWDGE (gpsimd) paths.
        YCH = 2 * RND  # steady-state y cols per DMA chunk
        ych_bounds = [0]
        if n_rounds:
            yw = n_rounds // 2 * MMF
            for sz in (MMF, MMF):
                if ych_bounds[-1] + sz < yw:
                    ych_bounds.append(ych_bounds[-1] + sz)
            while ych_bounds[-1] < yw:
                ych_bounds.append(min(yw, ych_bounds[-1] + YCH))

        def _t_of(c):
            for i in range(len(ych_bounds) - 1):
                if c < ych_bounds[i + 1]:
                    return i
            return len(ych_bounds) - 2

        # X goes out immediately on the SWDGE ring (DVE's SBUF-path work
        # depends on it); y streams concurrently on the HWDGE ring.
        X = xpool.tile([P, SLOTS, FP], DT_MY)
        for i in range(4):
            bounds = [0, 5, 9, 13, SLOTS]
            nc.gpsimd.dma_start(
                X[:, bounds[i] : bounds[i + 1], :], x_d[:, bounds[i] : bounds[i + 1], :]
            )

        ytiles = {}
        w = FP - PEF
        DSPLIT = 10  # slabs with d >= DSPLIT go DVE-sub + ACT-abs
        if sb:
            kmax = max(1, min(CHUNK // w, SLOTS - 2))
            sbpool = ctx.enter_context(tc.tile_pool(name="sbscratch", bufs=2))

        def emit_round(r):
            nonlocal col
            gp, wnd = r % 2, r // 2
            t = _t_of(wnd * MMF)
            if t not in ytiles:
                yt = ypool.tile([P, YCH], DT_MY, tag="yt", name="yt")
                nc.sync.dma_start(
                    yt[:, : ych_bounds[t + 1] - ych_bounds[t]],
                    y_d[:, ych_bounds[t] : ych_bounds[t + 1]],
                )
                ytiles[t] = yt
            yt = ytiles[t]
            yoff = wnd * MMF - ych_bounds[t]
            ps = pspool.tile([P, RND], F32, name="ps")
            for i, g in enumerate((2 * gp, 2 * gp + 1)):
                nc.tensor.matmul(
                    ps[:, i * MMF : (i + 1) * MMF],
                    Wt[32 * g : 32 * g + SLOTS, :],
                    yt[32 * g : 32 * g + SLOTS, yoff : yoff + MMF],
                    start=True,
                    stop=True,
                    tile_position=(32 * g, 0),
                )
            if r in act_rounds:
                aab = apool.tile([P, RND], DT_MY, tag="aab", name="aab")
                nc.scalar.activation(
                    aab[:], ps[:], mybir.ActivationFunctionType.Abs,
                    bias=zb[:],
                    accum_out=acc2[:, col : col + 1],
                )
            else:
                ab = spool.tile([P, RND], DT_MY, name="ab")
                nc.vector._custom_dve(
                    absacc, out=ab[:], in0=ps[:], accum_out=acc[:, col : col + 1]
                )
            meta.append("psum")
            col += 1

        def emit_chunk(chunk):
            nonlocal col
            kind, d, k0, kn = chunk
            if kind == 0:
                in0 = X[:, 0:1, PEF:FP]
                in1 = X[:, d : d + 1, PEF:FP]
                kn = 1
            else:
                in0 = X[:, k0 : k0 + kn, PEF:FP]
                in1 = X[:, k0 + d : k0 + d + kn, PEF:FP]
            if kind == 1 and d >= DSPLIT:
                df = dfpool.tile([P, SLOTS - DSPLIT, w], DT_MY, tag="df", name="df")
                nc.vector.tensor_sub(df[:, :kn, :], in0, in1)
                aab2 = apool.tile([P, SLOTS - DSPLIT, w], DT_MY, tag="aab2", name="aab2")
                nc.scalar.activation(
                    aab2[:, :kn, :], df[:, :kn, :],
                    mybir.ActivationFunctionType.Abs,
                    bias=zb[:],
                    accum_out=acc2[:, col : col + 1],
                )
            else:
                ab = sbpool.tile([P, kmax, w], DT_MY, name="ab_sb")
                nc.vector._custom_dve(
                    fused, out=ab[:, :kn, :], in0=in0, in1=in1,
                    accum_out=acc[:, col : col + 1],
                )
            meta.append(kind)
            col += 1

        def emit_leftover():
            nonlocal col
            ab = lpool.tile([P, LEFT_D, PEF], DT_MY, name="ab_left")
            nc.vector._custom_dve(
                fused,
                out=ab[:],
                in0=X[:, 1 : 1 + LEFT_D, :PEF],
                in1=X[:, 1 + LEFT_D : SLOTS, :PEF],
                accum_out=acc[:, col : col + 1],
            )
            meta.append(1)
            col += 1

        # Emission order = per-engine program order. ACT: ~19 rounds first
        # (only need y chunks), then the split-slab absreds (X-dependent),
        # then remaining rounds. DVE: its prefix rounds, split-slab subs,
        # leftover, fused slabs.
        split_sb = [c for c in sb if c[0] == 1 and c[1] >= DSPLIT]
        rest_sb = [c for c in sb if not (c[0] == 1 and c[1] >= DSPLIT)]
        R1 = min(25, n_rounds)
        for r in range(R1):
            emit_round(r)
        for c_ in split_sb:
            emit_chunk(c_)
        if PEF > 0:
            emit_leftover()
        for r in range(R1, n_rounds):
            emit_round(r)
        for c_ in rest_sb:
            emit_chunk(c_)

        nc.sync.dma_start(acc_d[:], acc[:])
        nc.sync.dma_start(acc2_d[:], acc2[:])

    nc.compile()
    return nc, meta


def _in_maps(preds, gt):
    preds = np.asarray(preds)
    gt = np.asarray(gt)
    pair_cols = _pair_columns()
    Wm = np.zeros((P, P), dtype=DT)
    for c, (i, j) in enumerate(pair_cols):
        for g in range(GROUPS):
            Wm[32 * g + i, c] = 1.0
            Wm[32 * g + j, c] = -1.0
    maps = []
    for b in range(B):
        X = np.empty((P, SLOTS, FP), dtype=DT)
        X[:, 0, :] = gt[b].reshape(P, FP)
        X[:, 1:, :] = preds[b].reshape(N, P, FP).transpose(1, 0, 2)
        m = {"x": X, "w": Wm}
        if PEF > 0:
            Y = np.zeros((P, 32 * PEF), dtype=DT)
            for g in range(GROUPS):
                blk = X[32 * g : 32 * (g + 1), :, :PEF]  # [32, 17, PEF]
                Y[32 * g : 32 * g + SLOTS] = blk.transpose(1, 0, 2).reshape(
                    SLOTS, 32 * PEF
                )
            m["y"] = Y
        maps.append(m)
    return maps


def _finish(results, meta):
    t1 = 0.0
    t2 = 0.0
    for r in results:
        a = r["acc"].astype(np.float64) + r["acc2"].astype(np.float64)
        for col, kind in enumerate(meta):
            if kind == "psum":
                t1 += a[:N, col].sum()
                t2 += a[N:, col].sum()
            elif kind == 0:
                t1 += a[:, col].sum()
            else:
                t2 += a[:, col].sum()
    val = (t1 / N - t2 / (N * (N - 1))) / (B * CHW)
    return np.float32(val)


def _run(preds, gt, trace=False, **kw):
    if "nc" not in _cache:
        _cache["nc"] = _build()
    nc, meta = _cache["nc"]
    res = run_bass_kernel_spmd(nc, _in_maps(preds, gt), list(range(8)), trace=trace, **kw)
    return _finish(res.results, meta), res


def kernel(preds, gt):
    out, _ = _run(preds, gt)
    return out


# revision 49
# speedup vs baseline: 1.0215x; 1.0215x over previous
"""CRPS loss kernel for Trainium2 (8 NeuronCores, axon-tunneled).

reference semantics:
  preds: [B=8, N=16, C=4, H=128, W=256] f32, gt: [B, C, H, W] f32
  term1 = mean_i |preds_i - gt|            (per point)
  term2 = sum_{i,j} |p_i - p_j| / (2N(N-1))
  out   = mean(term1 - term2)  (scalar f32)

Sharding: batch-parallel, core b handles batch element b (131072 points).

The 17 values per point (gt + 16 members) need |.| of 136 pair
differences (16 vs-gt + 120 member pairs). Engine plan per core:

 * TensorE path (points with c < PEF): a constant [17, 128] +1/-1
   stationary matrix turns one matmul into 128 pair-differences per
   point (16 term1 + 112 term2 columns; the 8 pairs (k, k+8) are left
   out to fit M=128). Moving data is streamed N-major; 4 copies of the
   stationary sit at partition 32g so row-group matmuls run
   concurrently. Each round fills a [128, 1024] 2-bank PSUM slot (two
   512-col matmuls on paired groups); a 4-slot ring keeps TensorE ~2
   slots ahead. ScalarE (activation Abs + accum_out) and VectorE
   (custom |x| + accum op) split the rounds so both consume in
   parallel at ~93% occupancy.
   The 8 leftover pairs come from the spatial-major copy via one big
   fused ABS_DIFF_ACC instruction.
 * SBUF path (points with c >= PEF): all 136 pairs via the fused
   ABS_DIFF_ACC custom DVE op (native ABSOLUTE_DIFF alu op + ADD
   accumulate) on slot-offset slices of the spatial-major tile.

Per-instruction partial sums land in separate columns of a [128, ncols]
f32 accumulator; the host finishes the reduction in float64. For PSUM
columns the partition index is the PAIR index (rows 0-15 = term1), for
SBUF columns it is spatial.

Inputs are cast to fp16 on the host (DVE tensor ops hit 2x mode, DMA
halves); all accumulation is fp32. Measured end-to-end scalar error vs
the f32 reference is ~1e-6 (noise cancels over 142M terms).
"""

import re
import sys

if "/opt/trn_rl_repo" not in sys.path:
    sys.path.insert(0, "/opt/trn_rl_repo")

from contextlib import ExitStack

import numpy as np

import concourse.tile as tile
from concourse import bacc, mybir
from concourse import dve_ops
from concourse.bass_utils import run_bass_kernel_spmd
from concourse.dve_spec import AluOp, Bin, Spec, Src0, Src1, Zero, maxx
from concourse.dve_ops import OPS, CUSTOM_DVE_SPECS, _SUB_OPCODE_FOR_NAME, DveOp

B, N, C, H, W = 8, 16, 4, 128, 256
CHW = C * H * W          # 131072 spatial points per batch element
P = 128                  # SBUF partitions
FP = CHW // P            # 1024 points per partition
SLOTS = N + 1            # gt + 16 members
XW = SLOTS * FP
CHUNK = 16384            # max free-dim per SBUF-path instruction

DT = np.float16
DT_MY = mybir.dt.float16
F32 = mybir.dt.float32

PEF = 512                # cols per partition routed through the TensorE path
MMF = 512                # moving free-dim per matmul
GROUPS = 4               # concurrent row-group matmuls
RND = 2 * MMF            # points consumed per PSUM round (2 groups/round)
PSUM_BUFS = 4

# leftover pairs not in the matmul: (slot k, slot k+8) for k=1..8
LEFT_D = 8

# measured per-round consumer costs (ns) to split rounds ACT vs DVE
_COST_ACT_ROUND = 1557.0
_COST_DVE_ROUND = 1390.0

_cache = {}


def _register_op(name, spec):
    for op in OPS:
        if op.name == name:
            return op
    op = DveOp(name, spec, subdim=False, uops_sha={})
    OPS.append(op)
    CUSTOM_DVE_SPECS[name] = spec
    _SUB_OPCODE_FOR_NAME[name] = dve_ops._CUSTOM_DVE_ROW_BASE + len(OPS) - 1
    for ver in ("v3", "v4"):
        try:
            op.compile(ver)
        except ValueError as e:
            m = re.search(r":\s*([0-9a-f]{8,32})\s*≠", str(e))
            if not m:
                raise
            op.uops_sha[ver] = m.group(1)
            op.compile(ver)
    return op


def _abs_diff_acc():
    """out = |in0 - in1|; accum_out = sum(out) along free dim."""
    return _register_op(
        "ABS_DIFF_ACC_CRPS",
        Spec(
            body=Bin(AluOp.ABSOLUTE_DIFF, Src0, Src1),
            accum=AluOp.ADD,
            reference=lambda in0, in1, s0, s1, imm2: np.abs(
                np.asarray(in0, np.float32) - np.asarray(in1, np.float32)
            ),
        ),
    )


def _abs_acc():
    """out = |in0|; accum_out = sum(out) along free dim (PSUM consumer)."""
    return _register_op(
        "ABS_ACC_CRPS",
        Spec(
            body=maxx(Src0, Zero - Src0),
            accum=AluOp.ADD,
            reference=lambda in0, in1, s0, s1, imm2: np.abs(
                np.asarray(in0, np.float32)
            ),
        ),
    )


def _pair_columns():
    """Stationary matrix columns: (pos_slot, neg_slot). First 16 = term1."""
    cols = [(0, d) for d in range(1, SLOTS)]
    for d in range(1, N):
        if d == LEFT_D:
            continue
        for k in range(1, SLOTS - d):
            cols.append((k, k + d))
    assert len(cols) == 128
    return cols


def _sbuf_chunks():
    """(kind, d, k0, kn) slot-window work units over cols [PEF, FP).

    kind 0: gt pairs (slot 0, slot d) — k0/kn unused (single window).
    kind 1: member pairs (slot k, slot k+d) for k in [k0, k0+kn).
    Chunks are whole slot windows; window width w = FP - PEF.
    """
    out = []
    w = FP - PEF
    if w <= 0:
        return out
    kmax = max(1, min(CHUNK // w, SLOTS - 2))
    for d in range(1, SLOTS):
        out.append((0, d, 0, 1))
        if d < SLOTS - 1:
            nk = SLOTS - 1 - d
            for k0 in range(1, 1 + nk, kmax):
                out.append((1, d, k0, min(kmax, 1 + nk - k0)))
    return out


def _build():
    fused = _abs_diff_acc()
    absacc = _abs_acc()
    nc = bacc.Bacc("TRN2", target_bir_lowering=False, debug=False, num_devices=8)

    n_rounds = (P * PEF) // RND
    x_d = nc.dram_tensor("x", [P, SLOTS, FP], DT_MY, kind="ExternalInput").ap()
    w_d = nc.dram_tensor("w", [P, P], DT_MY, kind="ExternalInput").ap()
    y_d = (
        nc.dram_tensor("y", [P, 32 * PEF], DT_MY, kind="ExternalInput").ap()
        if n_rounds
        else None
    )

    sb = _sbuf_chunks()
    # columns: [psum rounds][leftover d=8][sbuf chunks]
    ncols = n_rounds + 1 + len(sb)
    acc_d = nc.dram_tensor("acc", [P, ncols], F32, kind="ExternalOutput").ap()

    # round -> consumer engine, balancing measured costs. DVE's SBUF-path
    # work needs X (which lands ~20us in), so DVE takes the FIRST rounds
    # and ACT everything after.
    left_cost = (LEFT_D * PEF) / 0.96 + 300.0
    w_sb = FP - PEF
    sbuf_cost = sum(kn * w_sb for _, _, _, kn in sb) / 0.96 + len(sb) * 160.0
    dve_fixed = left_cost + sbuf_cost
    n_act = n_rounds
    if n_rounds:
        n_act = max(0, min(n_rounds, round((_COST_DVE_ROUND * n_rounds + dve_fixed)
                                           / (_COST_ACT_ROUND + _COST_DVE_ROUND))))
    act_rounds = set(range(n_rounds - n_act, n_rounds))

    with tile.TileContext(nc) as tc, ExitStack() as ctx:
        xpool = ctx.enter_context(tc.tile_pool(name="x", bufs=1))
        wpool = ctx.enter_context(tc.tile_pool(name="w", bufs=1))
        ypool = ctx.enter_context(tc.tile_pool(name="y", bufs=4))
        pspool = ctx.enter_context(tc.tile_pool(name="ps", bufs=PSUM_BUFS, space="PSUM"))
        spool = ctx.enter_context(tc.tile_pool(name="scratch", bufs=2))
        apool = ctx.enter_context(tc.tile_pool(name="ascratch", bufs=2))
        dfpool = ctx.enter_context(tc.tile_pool(name="dfscratch", bufs=2))
        lpool = ctx.enter_context(tc.tile_pool(name="lscratch", bufs=2))
        accpool = ctx.enter_context(tc.tile_pool(name="acc", bufs=1))

        Wt = wpool.tile([P, P], DT_MY)
        nc.sync.dma_start(Wt[:], w_d[:])
        acc = accpool.tile([P, ncols], F32)
        # explicit zero bias for Abs activations: the implicit const AP's
        # init otherwise lands behind the big X transfer in queue order,
        # stalling ACT's first instruction ~15us
        zb = accpool.tile([P, 1], F32, name="zb")
        nc.vector.memset(zb[:], 0.0)

        col = 0
        meta = []  # per-col: "psum" | 0 (term1) | 1 (term2)

        # --- TensorE path ---
        # y chunks stream ahead of the matmuls; the big X transfer is issued
        # interleaved late (leftover/SBUF work needs it last) so it doesn't
        # steal HBM bandwidth from the critical y stream. Chunk DMAs
        # alternate between the HWDGE (sync) and SWDGE (gpsimd) paths.
        YCH = 2 * RND  # steady-state y cols per DMA chunk
        ych_bounds = [0]
        if n_rounds:
            yw = n_rounds // 2 * MMF
            for sz in (MMF, MMF):
                if ych_bounds[-1] + sz < yw:
                    ych_bounds.append(ych_bounds[-1] + sz)
            while ych_bounds[-1] < yw:
                ych_bounds.append(min(yw, ych_bounds[-1] + YCH))

        def _t_of(c):
            for i in range(len(ych_bounds) - 1):
                if c < ych_bounds[i + 1]:
                    return i
            return len(ych_bounds) - 2

        # X goes out immediately on the SWDGE ring (DVE's SBUF-path work
        # depends on it); y streams concurrently on the HWDGE ring.
        X = xpool.tile([P, SLOTS, FP], DT_MY)
        for i in range(4):
            bounds = [0, 5, 9, 13, SLOTS]
            nc.gpsimd.dma_start(
                X[:, bounds[i] : bounds[i + 1], :], x_d[:, bounds[i] : bounds[i + 1], :]
            )

        ytiles = {}
        w = FP - PEF
        DSPLIT = 10  # slabs with d >= DSPLIT go DVE-sub + ACT-abs
        if sb:
            kmax = max(1, min(CHUNK // w, SLOTS - 2))
            sbpool = ctx.enter_context(tc.tile_pool(name="sbscratch", bufs=2))

        def emit_round(r):
            nonlocal col
            gp, wnd = r % 2, r // 2
            t = _t_of(wnd * MMF)
            if t not in ytiles:
                yt = ypool.tile([P, YCH], DT_MY, tag="yt", name="yt")
                nc.sync.dma_start(
                    yt[:, : ych_bounds[t + 1] - ych_bounds[t]],
                    y_d[:, ych_bounds[t] : ych_bounds[t + 1]],
                )
                ytiles[t] = yt
            yt = ytiles[t]
            yoff = wnd * MMF - ych_bounds[t]
            ps = pspool.tile([P, RND], F32, name="ps")
            for i, g in enumerate((2 * gp, 2 * gp + 1)):
                nc.tensor.matmul(
                    ps[:, i * MMF : (i + 1) * MMF],
                    Wt[32 * g : 32 * g + SLOTS, :],
                    yt[32 * g : 32 * g + SLOTS, yoff : yoff + MMF],
                    start=True,
                    stop=True,
                    tile_position=(32 * g, 0),
                )
            if r in act_rounds:
                aab = apool.tile([P, RND], DT_MY, tag="aab", name="aab")
                nc.scalar.activation(
                    aab[:], ps[:], mybir.ActivationFunctionType.Abs,
                    bias=zb[:],
                    accum_out=acc[:, col : col + 1],
                )
            else:
                ab = spool.tile([P, RND], DT_MY, name="ab")
                nc.vector._custom_dve(
                    absacc, out=ab[:], in0=ps[:], accum_out=acc[:, col : col + 1]
                )
            meta.append("psum")
            col += 1

        def emit_chunk(chunk):
            nonlocal col
            kind, d, k0, kn = chunk
            if kind == 0:
                in0 = X[:, 0:1, PEF:FP]
                in1 = X[:, d : d + 1, PEF:FP]
                kn = 1
            else:
                in0 = X[:, k0 : k0 + kn, PEF:FP]
                in1 = X[:, k0 + d : k0 + d + kn, PEF:FP]
            if kind == 1 and d >= DSPLIT:
                df = dfpool.tile([P, SLOTS - DSPLIT, w], DT_MY, tag="df", name="df")
                nc.vector.tensor_sub(df[:, :kn, :], in0, in1)
                aab2 = apool.tile([P, SLOTS - DSPLIT, w], DT_MY, tag="aab2", name="aab2")
                nc.scalar.activation(
                    aab2[:, :kn, :], df[:, :kn, :],
                    mybir.ActivationFunctionType.Abs,
                    bias=zb[:],
                    accum_out=acc[:, col : col + 1],
                )
            else:
                ab = sbpool.tile([P, kmax, w], DT_MY, name="ab_sb")
                nc.vector._custom_dve(
                    fused, out=ab[:, :kn, :], in0=in0, in1=in1,
                    accum_out=acc[:, col : col + 1],
                )
            meta.append(kind)
            col += 1

        def emit_leftover():
            nonlocal col
            ab = lpool.tile([P, LEFT_D, PEF], DT_MY, name="ab_left")
            nc.vector._custom_dve(
                fused,
                out=ab[:],
                in0=X[:, 1 : 1 + LEFT_D, :PEF],
                in1=X[:, 1 + LEFT_D : SLOTS, :PEF],
                accum_out=acc[:, col : col + 1],
            )
            meta.append(1)
            col += 1

        # Emission order = per-engine program order. ACT: ~19 rounds first
        # (only need y chunks), then the split-slab absreds (X-dependent),
        # then remaining rounds. DVE: its prefix rounds, split-slab subs,
        # leftover, fused slabs.
        split_sb = [c for c in sb if c[0] == 1 and c[1] >= DSPLIT]
        rest_sb = [c for c in sb if not (c[0] == 1 and c[1] >= DSPLIT)]
        R1 = min(25, n_rounds)
        for r in range(R1):
            emit_round(r)
        for c_ in split_sb:
            emit_chunk(c_)
        if PEF > 0:
            emit_leftover()
        for r in range(R1, n_rounds):
            emit_round(r)
        for c_ in rest_sb:
            emit_chunk(c_)

        nc.sync.dma_start(acc_d[:], acc[:])

    nc.compile()
    return nc, meta


def _in_maps(preds, gt):
    preds = np.asarray(preds)
    gt = np.asarray(gt)
    pair_cols = _pair_columns()
    Wm = np.zeros((P, P), dtype=DT)
    for c, (i, j) in enumerate(pair_cols):
        for g in range(GROUPS):
            Wm[32 * g + i, c] = 1.0
            Wm[32 * g + j, c] = -1.0
    maps = []
    for b in range(B):
        X = np.empty((P, SLOTS, FP), dtype=DT)
        X[:, 0, :] = gt[b].reshape(P, FP)
        X[:, 1:, :] = preds[b].reshape(N, P, FP).transpose(1, 0, 2)
        m = {"x": X, "w": Wm}
        if PEF > 0:
            Y = np.zeros((P, 32 * PEF), dtype=DT)
            for g in range(GROUPS):
                blk = X[32 * g : 32 * (g + 1), :, :PEF]  # [32, 17, PEF]
                Y[32 * g : 32 * g + SLOTS] = blk.transpose(1, 0, 2).reshape(
                    SLOTS, 32 * PEF
                )
            m["y"] = Y
        maps.append(m)
    return maps


def _finish(results, meta):
    t1 = 0.0
    t2 = 0.0
    for r in results:
        a = r["acc"].astype(np.float64)
        for col, kind in enumerate(meta):
            if kind == "psum":
                t1 += a[:N, col].sum()
                t2 += a[N:, col].sum()
            elif kind == 0:
                t1 += a[:, col].sum()
            else:
                t2 += a[:, col].sum()
    val = (t1 / N - t2 / (N * (N - 1))) / (B * CHW)
    return np.float32(val)


def _run(preds, gt, trace=False, **kw):
    if "nc" not in _cache:
        _cache["nc"] = _build()
    nc, meta = _cache["nc"]
    res = run_bass_kernel_spmd(nc, _in_maps(preds, gt), list(range(8)), trace=trace, **kw)
    return _finish(res.results, meta), res


def kernel(preds, gt):
    out, _ = _run(preds, gt)
    return out


# revision 50
# speedup vs baseline: 1.0249x; 1.0034x over previous
"""CRPS loss kernel for Trainium2 (8 NeuronCores, axon-tunneled).

reference semantics:
  preds: [B=8, N=16, C=4, H=128, W=256] f32, gt: [B, C, H, W] f32
  term1 = mean_i |preds_i - gt|            (per point)
  term2 = sum_{i,j} |p_i - p_j| / (2N(N-1))
  out   = mean(term1 - term2)  (scalar f32)

Sharding: batch-parallel, core b handles batch element b (131072 points).

The 17 values per point (gt + 16 members) need |.| of 136 pair
differences (16 vs-gt + 120 member pairs). Engine plan per core:

 * TensorE path (points with c < PEF): a constant [17, 128] +1/-1
   stationary matrix turns one matmul into 128 pair-differences per
   point (16 term1 + 112 term2 columns; the 8 pairs (k, k+8) are left
   out to fit M=128). Moving data is streamed N-major; 4 copies of the
   stationary sit at partition 32g so row-group matmuls run
   concurrently. Each round fills a [128, 1024] 2-bank PSUM slot (two
   512-col matmuls on paired groups); a 4-slot ring keeps TensorE ~2
   slots ahead. ScalarE (activation Abs + accum_out) and VectorE
   (custom |x| + accum op) split the rounds so both consume in
   parallel at ~93% occupancy.
   The 8 leftover pairs come from the spatial-major copy via one big
   fused ABS_DIFF_ACC instruction.
 * SBUF path (points with c >= PEF): all 136 pairs via the fused
   ABS_DIFF_ACC custom DVE op (native ABSOLUTE_DIFF alu op + ADD
   accumulate) on slot-offset slices of the spatial-major tile.

Per-instruction partial sums land in separate columns of a [128, ncols]
f32 accumulator; the host finishes the reduction in float64. For PSUM
columns the partition index is the PAIR index (rows 0-15 = term1), for
SBUF columns it is spatial.

Inputs are cast to fp16 on the host (DVE tensor ops hit 2x mode, DMA
halves); all accumulation is fp32. Measured end-to-end scalar error vs
the f32 reference is ~1e-6 (noise cancels over 142M terms).
"""

import re
import sys

if "/opt/trn_rl_repo" not in sys.path:
    sys.path.insert(0, "/opt/trn_rl_repo")

from contextlib import ExitStack

import numpy as np

import concourse.tile as tile
from concourse import bacc, mybir
from concourse import dve_ops
from concourse.bass_utils import run_bass_kernel_spmd
from concourse.dve_spec import AluOp, Bin, Spec, Src0, Src1, Zero, maxx
from concourse.dve_ops import OPS, CUSTOM_DVE_SPECS, _SUB_OPCODE_FOR_NAME, DveOp

B, N, C, H, W = 8, 16, 4, 128, 256
CHW = C * H * W          # 131072 spatial points per batch element
P = 128                  # SBUF partitions
FP = CHW // P            # 1024 points per partition
SLOTS = N + 1            # gt + 16 members
XW = SLOTS * FP
CHUNK = 16384            # max free-dim per SBUF-path instruction

DT = np.float16
DT_MY = mybir.dt.float16
F32 = mybir.dt.float32

PEF = 512                # cols per partition routed through the TensorE path
MMF = 512                # moving free-dim per matmul
GROUPS = 4               # concurrent row-group matmuls
RND = 2 * MMF            # points consumed per PSUM round (2 groups/round)
PSUM_BUFS = 4

# leftover pairs not in the matmul: (slot k, slot k+8) for k=1..8
LEFT_D = 8

# measured per-round consumer costs (ns) to split rounds ACT vs DVE
_COST_ACT_ROUND = 1557.0
_COST_DVE_ROUND = 1390.0

_cache = {}


def _register_op(name, spec):
    for op in OPS:
        if op.name == name:
            return op
    op = DveOp(name, spec, subdim=False, uops_sha={})
    OPS.append(op)
    CUSTOM_DVE_SPECS[name] = spec
    _SUB_OPCODE_FOR_NAME[name] = dve_ops._CUSTOM_DVE_ROW_BASE + len(OPS) - 1
    for ver in ("v3", "v4"):
        try:
            op.compile(ver)
        except ValueError as e:
            m = re.search(r":\s*([0-9a-f]{8,32})\s*≠", str(e))
            if not m:
                raise
            op.uops_sha[ver] = m.group(1)
            op.compile(ver)
    return op


def _abs_diff_acc():
    """out = |in0 - in1|; accum_out = sum(out) along free dim."""
    return _register_op(
        "ABS_DIFF_ACC_CRPS",
        Spec(
            body=Bin(AluOp.ABSOLUTE_DIFF, Src0, Src1),
            accum=AluOp.ADD,
            reference=lambda in0, in1, s0, s1, imm2: np.abs(
                np.asarray(in0, np.float32) - np.asarray(in1, np.float32)
            ),
        ),
    )


def _abs_acc():
    """out = |in0|; accum_out = sum(out) along free dim (PSUM consumer)."""
    return _register_op(
        "ABS_ACC_CRPS",
        Spec(
            body=maxx(Src0, Zero - Src0),
            accum=AluOp.ADD,
            reference=lambda in0, in1, s0, s1, imm2: np.abs(
                np.asarray(in0, np.float32)
            ),
        ),
    )


def _pair_columns():
    """Stationary matrix columns: (pos_slot, neg_slot). First 16 = term1."""
    cols = [(0, d) for d in range(1, SLOTS)]
    for d in range(1, N):
        if d == LEFT_D:
            continue
        for k in range(1, SLOTS - d):
            cols.append((k, k + d))
    assert len(cols) == 128
    return cols


def _sbuf_chunks():
    """(kind, d, k0, kn) slot-window work units over cols [PEF, FP).

    kind 0: gt pairs (slot 0, slot d) — k0/kn unused (single window).
    kind 1: member pairs (slot k, slot k+d) for k in [k0, k0+kn).
    Chunks are whole slot windows; window width w = FP - PEF.
    """
    out = []
    w = FP - PEF
    if w <= 0:
        return out
    kmax = max(1, min(CHUNK // w, SLOTS - 2))
    for d in range(1, SLOTS):
        out.append((0, d, 0, 1))
        if d < SLOTS - 1:
            nk = SLOTS - 1 - d
            for k0 in range(1, 1 + nk, kmax):
                out.append((1, d, k0, min(kmax, 1 + nk - k0)))
    return out


def _build():
    fused = _abs_diff_acc()
    absacc = _abs_acc()
    nc = bacc.Bacc("TRN2", target_bir_lowering=False, debug=False, num_devices=8)

    n_rounds = (P * PEF) // RND
    x_d = nc.dram_tensor("x", [P, SLOTS, FP], DT_MY, kind="ExternalInput").ap()
    w_d = nc.dram_tensor("w", [P, P], DT_MY, kind="ExternalInput").ap()
    y_d = (
        nc.dram_tensor("y", [P, 32 * PEF], DT_MY, kind="ExternalInput").ap()
        if n_rounds
        else None
    )

    sb = _sbuf_chunks()
    # columns: [psum rounds][leftover d=8][sbuf chunks]
    ncols = n_rounds + 1 + len(sb)
    acc_d = nc.dram_tensor("acc", [P, ncols], F32, kind="ExternalOutput").ap()

    # round -> consumer engine, balancing measured costs. DVE's SBUF-path
    # work needs X (which lands ~20us in), so DVE takes the FIRST rounds
    # and ACT everything after.
    left_cost = (LEFT_D * PEF) / 0.96 + 300.0
    w_sb = FP - PEF
    sbuf_cost = sum(kn * w_sb for _, _, _, kn in sb) / 0.96 + len(sb) * 160.0
    dve_fixed = left_cost + sbuf_cost
    n_act = n_rounds
    if n_rounds:
        n_act = max(0, min(n_rounds, round((_COST_DVE_ROUND * n_rounds + dve_fixed)
                                           / (_COST_ACT_ROUND + _COST_DVE_ROUND))))
    # interleave the early rounds (ACT evens, DVE odds) so both engines
    # consume from round 0 — a contiguous DVE prefix stalls PE on ring
    # slots and delays ACT's first available round by ~8us
    n_dve = n_rounds - n_act
    act_rounds = set(range(0, 2 * n_dve, 2)) | set(range(2 * n_dve, n_rounds))

    with tile.TileContext(nc) as tc, ExitStack() as ctx:
        xpool = ctx.enter_context(tc.tile_pool(name="x", bufs=1))
        wpool = ctx.enter_context(tc.tile_pool(name="w", bufs=1))
        ypool = ctx.enter_context(tc.tile_pool(name="y", bufs=4))
        pspool = ctx.enter_context(tc.tile_pool(name="ps", bufs=PSUM_BUFS, space="PSUM"))
        spool = ctx.enter_context(tc.tile_pool(name="scratch", bufs=2))
        apool = ctx.enter_context(tc.tile_pool(name="ascratch", bufs=2))
        dfpool = ctx.enter_context(tc.tile_pool(name="dfscratch", bufs=2))
        lpool = ctx.enter_context(tc.tile_pool(name="lscratch", bufs=2))
        accpool = ctx.enter_context(tc.tile_pool(name="acc", bufs=1))

        Wt = wpool.tile([P, P], DT_MY)
        nc.sync.dma_start(Wt[:], w_d[:])
        acc = accpool.tile([P, ncols], F32)
        # explicit zero bias for Abs activations: the implicit const AP's
        # init otherwise lands behind the big X transfer in queue order,
        # stalling ACT's first instruction ~15us
        zb = accpool.tile([P, 1], F32, name="zb")
        nc.vector.memset(zb[:], 0.0)

        col = 0
        meta = []  # per-col: "psum" | 0 (term1) | 1 (term2)

        # --- TensorE path ---
        # y chunks stream ahead of the matmuls; the big X transfer is issued
        # interleaved late (leftover/SBUF work needs it last) so it doesn't
        # steal HBM bandwidth from the critical y stream. Chunk DMAs
        # alternate between the HWDGE (sync) and SWDGE (gpsimd) paths.
        YCH = 2 * RND  # steady-state y cols per DMA chunk
        ych_bounds = [0]
        if n_rounds:
            yw = n_rounds // 2 * MMF
            for sz in (MMF, MMF):
                if ych_bounds[-1] + sz < yw:
                    ych_bounds.append(ych_bounds[-1] + sz)
            while ych_bounds[-1] < yw:
                ych_bounds.append(min(yw, ych_bounds[-1] + YCH))

        def _t_of(c):
            for i in range(len(ych_bounds) - 1):
                if c < ych_bounds[i + 1]:
                    return i
            return len(ych_bounds) - 2

        # X goes out immediately on the SWDGE ring (DVE's SBUF-path work
        # depends on it); y streams concurrently on the HWDGE ring.
        X = xpool.tile([P, SLOTS, FP], DT_MY)
        for i in range(4):
            bounds = [0, 5, 9, 13, SLOTS]
            nc.gpsimd.dma_start(
                X[:, bounds[i] : bounds[i + 1], :], x_d[:, bounds[i] : bounds[i + 1], :]
            )

        ytiles = {}
        w = FP - PEF
        DSPLIT = 10  # slabs with d >= DSPLIT go DVE-sub + ACT-abs
        if sb:
            kmax = max(1, min(CHUNK // w, SLOTS - 2))
            sbpool = ctx.enter_context(tc.tile_pool(name="sbscratch", bufs=2))

        def emit_round(r):
            nonlocal col
            gp, wnd = r % 2, r // 2
            t = _t_of(wnd * MMF)
            if t not in ytiles:
                yt = ypool.tile([P, YCH], DT_MY, tag="yt", name="yt")
                nc.sync.dma_start(
                    yt[:, : ych_bounds[t + 1] - ych_bounds[t]],
                    y_d[:, ych_bounds[t] : ych_bounds[t + 1]],
                )
                ytiles[t] = yt
            yt = ytiles[t]
            yoff = wnd * MMF - ych_bounds[t]
            ps = pspool.tile([P, RND], F32, name="ps")
            for i, g in enumerate((2 * gp, 2 * gp + 1)):
                nc.tensor.matmul(
                    ps[:, i * MMF : (i + 1) * MMF],
                    Wt[32 * g : 32 * g + SLOTS, :],
                    yt[32 * g : 32 * g + SLOTS, yoff : yoff + MMF],
                    start=True,
                    stop=True,
                    tile_position=(32 * g, 0),
                )
            if r in act_rounds:
                aab = apool.tile([P, RND], DT_MY, tag="aab", name="aab")
                nc.scalar.activation(
                    aab[:], ps[:], mybir.ActivationFunctionType.Abs,
                    bias=zb[:],
                    accum_out=acc[:, col : col + 1],
                )
            else:
                ab = spool.tile([P, RND], DT_MY, name="ab")
                nc.vector._custom_dve(
                    absacc, out=ab[:], in0=ps[:], accum_out=acc[:, col : col + 1]
                )
            meta.append("psum")
            col += 1

        def emit_chunk(chunk):
            nonlocal col
            kind, d, k0, kn = chunk
            if kind == 0:
                in0 = X[:, 0:1, PEF:FP]
                in1 = X[:, d : d + 1, PEF:FP]
                kn = 1
            else:
                in0 = X[:, k0 : k0 + kn, PEF:FP]
                in1 = X[:, k0 + d : k0 + d + kn, PEF:FP]
            if kind == 1 and d >= DSPLIT:
                df = dfpool.tile([P, SLOTS - DSPLIT, w], DT_MY, tag="df", name="df")
                nc.vector.tensor_sub(df[:, :kn, :], in0, in1)
                aab2 = apool.tile([P, SLOTS - DSPLIT, w], DT_MY, tag="aab2", name="aab2")
                nc.scalar.activation(
                    aab2[:, :kn, :], df[:, :kn, :],
                    mybir.ActivationFunctionType.Abs,
                    bias=zb[:],
                    accum_out=acc[:, col : col + 1],
                )
            else:
                ab = sbpool.tile([P, kmax, w], DT_MY, name="ab_sb")
                nc.vector._custom_dve(
                    fused, out=ab[:, :kn, :], in0=in0, in1=in1,
                    accum_out=acc[:, col : col + 1],
                )
            meta.append(kind)
            col += 1

        def emit_leftover():
            nonlocal col
            ab = lpool.tile([P, LEFT_D, PEF], DT_MY, name="ab_left")
            nc.vector._custom_dve(
                fused,
                out=ab[:],
                in0=X[:, 1 : 1 + LEFT_D, :PEF],
                in1=X[:, 1 + LEFT_D : SLOTS, :PEF],
                accum_out=acc[:, col : col + 1],
            )
            meta.append(1)
            col += 1

        # Emission order = per-engine program order. ACT: ~19 rounds first
        # (only need y chunks), then the split-slab absreds (X-dependent),
        # then remaining rounds. DVE: its prefix rounds, split-slab subs,
        # leftover, fused slabs.
        split_sb = [c for c in sb if c[0] == 1 and c[1] >= DSPLIT]
        rest_sb = [c for c in sb if not (c[0] == 1 and c[1] >= DSPLIT)]
        R1 = min(25, n_rounds)
        for r in range(R1):
            emit_round(r)
        for c_ in split_sb:
            emit_chunk(c_)
        if PEF > 0:
            emit_leftover()
        for r in range(R1, n_rounds):
            emit_round(r)
        for c_ in rest_sb:
            emit_chunk(c_)

        nc.sync.dma_start(acc_d[:], acc[:])

    nc.compile()
    return nc, meta


def _in_maps(preds, gt):
    preds = np.asarray(preds)
    gt = np.asarray(gt)
    pair_cols = _pair_columns()
    Wm = np.zeros((P, P), dtype=DT)
    for c, (i, j) in enumerate(pair_cols):
        for g in range(GROUPS):
            Wm[32 * g + i, c] = 1.0
            Wm[32 * g + j, c] = -1.0
    maps = []
    for b in range(B):
        X = np.empty((P, SLOTS, FP), dtype=DT)
        X[:, 0, :] = gt[b].reshape(P, FP)
        X[:, 1:, :] = preds[b].reshape(N, P, FP).transpose(1, 0, 2)
        m = {"x": X, "w": Wm}
        if PEF > 0:
            Y = np.zeros((P, 32 * PEF), dtype=DT)
            for g in range(GROUPS):
                blk = X[32 * g : 32 * (g + 1), :, :PEF]  # [32, 17, PEF]
                Y[32 * g : 32 * g + SLOTS] = blk.transpose(1, 0, 2).reshape(
                    SLOTS, 32 * PEF
                )
            m["y"] = Y
        maps.append(m)
    return maps


def _finish(results, meta):
    t1 = 0.0
    t2 = 0.0
    for r in results:
        a = r["acc"].astype(np.float64)
        for col, kind in enumerate(meta):
            if kind == "psum":
                t1 += a[:N, col].sum()
                t2 += a[N:, col].sum()
            elif kind == 0:
                t1 += a[:, col].sum()
            else:
                t2 += a[:, col].sum()
    val = (t1 / N - t2 / (N * (N - 1))) / (B * CHW)
    return np.float32(val)


def _run(preds, gt, trace=False, **kw):
    if "nc" not in _cache:
        _cache["nc"] = _build()
    nc, meta = _cache["nc"]
    res = run_bass_kernel_spmd(nc, _in_maps(preds, gt), list(range(8)), trace=trace, **kw)
    return _finish(res.results, meta), res


def kernel(preds, gt):
    out, _ = _run(preds, gt)
    return out
